# revision 52
# baseline (speedup 1.0000x reference)
"""Multi-head attention (B=2, L=2048, D=2048, 16 heads x 128) on 8 trn2 cores.

Sharding: tensor-parallel over heads (4 groups of 4 heads) x data-parallel
over batch (2) -> 8 cores.  Each core computes, for its (batch b, group g):
    hq = q_b @ Wq_g.T, hk = kv_b @ Wk_g.T, hv = kv_b @ Wv_g.T   (4 heads)
    per head: P = softmax(hq hk^T / sqrt(128)), o = P hv
    partial_out = concat_heads(o) @ Wo[:, g].T        [2048, 2048]
Host sums the 4 per-group partials for each batch.

Precision: projections stream bf16 x/w from HBM (halves DMA, keeps the
proj phase under the per-core HBM roofline); hq/hk/hv and the scores/exp/
AV chain stay float32r (TF32-like, full PE rate at free-dim 512); the Wo
path (o, Wo, staged output) is bf16.  Measured end-to-end max-rel ~5e-3.

Per-core schedule (all matmuls bf16, free-dim 512 = 1 PSUM bank, 1 cyc/col
at 2.4GHz warm = 216ns spacing):
  warm-up: ~9 throwaway matmuls on uninitialized SBUF ramp the HAM clock
    gate (1.2 -> 2.4 GHz needs ~3.4us of sustained PE activity) while the
    first DMAs cover their ~3us queue-startup latency.
  projections: 3 passes x q-blocks x 16 contraction chunks x 4 heads.
    DMA model learned from traces: all queues stripe over the same 16
    engines (~340GB/s aggregate, ~170 per active queue), each queue has an
    8-deep completion-sem rotation (more outstanding issues stall the
    issuing ENGINE), and an issue can also stall on tile-WAR waits.  So:
    (pass0, n0) x superblocks are split per chunk-pair (8 issues on sync,
    in consumption order), w chunks stream on gpsimd; later blocks
    alternate the x stream between the sync and scalar queues; next-pass
    weights prefetch as a gpsimd burst at pass0-n3 / pass1-n2 where that
    queue is otherwise quiet.  Q-projection q-block 1 is deferred into
    the attention phase.
  attention per (q-block n, head h), 11 steps:
    steps 0-7: scores pair p (2 matmuls kt=2p,2p+1 -> PSUM pp0) -> exp on
    ACT into bf16 SBUF; steps 3-10: AV pair p-3 (PE accumulates o^T in
    pp1; the 3-pair lag keeps ACT's exp off the PE critical path).
    Softmax denominator: DVE binary-tree sums the bf16 exp tiles (the
    tiles gated on the last exp form the shortest chain; the LAST
    iteration reorders the tree so only 2 adds trail the last exp); one
    PE ones-matmul folds partitions, deferred into the NEXT iteration's
    step 2 so the PE never waits on the DVE tree; DVE reciprocal +
    scale -> o_sb (bf16).
    n==0: steps {3,5,7,9} run 4 deferred Q-projection matmuls each
    (fills the otherwise ACT-bound first block; hq block 1 is copied
    out on DVE at iteration end, one head per iteration).
    n>0: those slots emit Wo groups for block n-1 (4 matmuls into pp2,
    bf16 stage alternating ACT/DVE, DMA out on sync -- a gpsimd-queue
    transfer near the kernel end makes the epilogue's gpsimd DRAIN
    stall ~8us).
  tail: last block's Wo as 16 512-wide groups, accumulators cycling the
    pp2/pp1/pp0 one-bank slots (6 in flight).  Cross-engine deps are
    tile-granular, so o reads emitted after the final flush's mul would
    wait on it: the fold+recip run after group 0's h0-h2, five more
    group-heads run pre-mul, and each group's h3 matmul trails 6 groups.
    The last two groups split casts over ACT+DVE and their output DMAs
    in half so the final write lands right behind the last matmul (the
    epilogue's fixed ~8us barrier + sem-reset sweep starts at queue
    drain).
"""
import math
import sys

for _p in ("/opt/trn_rl_repo", "/root/.axon_site/_ro/trn_rl_repo"):
    if _p not in sys.path:
        sys.path.append(_p)

import numpy as np

B = 2
L = 2048           # LQ == LK
DIN = 2048
NH = 16            # total heads
HL = 4             # heads per core
D = 128            # head dim
HD = HL * D        # 512, head-group width
DOUT = 2048
NC_ = 8            # cores
NCH = DIN // 128   # 16 contraction chunks
NQ = 4             # q blocks of 512
QB = 512
NKT = L // 128     # 16 key tiles

_CACHE = {}


def _build_nc():
    import concourse.bacc as bacc
    import concourse.mybir as mybir
    import concourse.tile as tile

    F32R = mybir.dt.float32r
    F32 = mybir.dt.float32
    BF16 = mybir.dt.bfloat16

    nc = bacc.Bacc("TRN2", target_bir_lowering=False, debug=False)
    qT = nc.dram_tensor("qT", [DIN, L], BF16, kind="ExternalInput").ap()
    kvT = nc.dram_tensor("kvT", [DIN, L], BF16, kind="ExternalInput").ap()
    wqT = nc.dram_tensor("wqT", [DIN, HD], BF16, kind="ExternalInput").ap()
    wkT = nc.dram_tensor("wkT", [DIN, HD], BF16, kind="ExternalInput").ap()
    wvT = nc.dram_tensor("wvT", [DIN, HD], BF16, kind="ExternalInput").ap()
    woT = nc.dram_tensor("woT", [HD, DOUT], BF16, kind="ExternalInput").ap()
    allones = nc.dram_tensor("allones", [128, 128], BF16, kind="ExternalInput").ap()
    out = nc.dram_tensor("out", [L, DOUT], BF16, kind="ExternalOutput").ap()

    EXP = mybir.ActivationFunctionType.Exp
    COPY = mybir.ActivationFunctionType.Copy

    with tile.TileContext(nc) as tc:
        with (
            nc.allow_low_precision(reason="bf16 io + fp32r attention core"),
            tc.tile_pool(name="persist", bufs=1) as pp,
            tc.tile_pool(name="psum", bufs=2, space="PSUM") as psp,
        ):
            hq_sb = pp.tile([128, HL * L], BF16, tag="hq")
            hk_sb = pp.tile([128, HL * L], BF16, tag="hk")
            hv_sb = pp.tile([128, NKT * HD], BF16, tag="hv")
            ones_sb = pp.tile([128, 128], BF16, tag="ones")

            # PE warm-up: the HAM clock gate runs the PE at 1.2 GHz until
            # ~3.4us of sustained activity, and the first real matmul can't
            # start until the first weight/x DMAs land (~4us after the
            # engines enter the kernel).  Throwaway matmuls on (uninitialized)
            # SBUF ramp the clock during that DMA wait so real work starts
            # at 2.4 GHz.
            warm_ps = psp.tile([128, QB], F32, tag="pp1", name="warm")
            for wi in range(9):
                nc.tensor.matmul(
                    warm_ps[:],
                    hq_sb[:, wi * 128 : (wi + 1) * 128],
                    hq_sb[:, 0:QB],
                    start=True,
                    stop=True,
                )

            # ---------------- projections ----------------
            # Q-block n=1 is deferred into the attention phase (its matmuls
            # fill the otherwise ACT-bound first attention block), so wq
            # tiles persist and w tags triple-buffer across the 3 passes.
            with tc.tile_pool(name="proj", bufs=1) as jp:
                w_drams = [wqT, wkT, wvT]
                w_tiles = {}

                def w_dma(pass_i, c, queue=None):
                    t = pp.tile(
                        [128, HD], BF16, tag="w", bufs=48, name=f"w{pass_i}_{c}"
                    )
                    (queue or nc.gpsimd).dma_start(
                        out=t[:], in_=w_drams[pass_i][c * 128 : (c + 1) * 128, :]
                    )
                    w_tiles[(pass_i, c)] = t

                def x_sblk(x_dram, cs, n, queue, split=False):
                    # x superblocks: 4 contraction chunks x one q block.  bufs=6
                    # so a new block's DMA never WARs a buffer freed only at
                    # the end of the previous block.  split=True issues one DMA
                    # per chunk-PAIR (same tile) so the first matmuls wait on
                    # 256KB, not 512KB -- used where the PE runs right at the
                    # DMA leading edge (pair grain: per-chunk grain doubles the
                    # ~650ns issue instructions, which starves the queue).
                    t = jp.tile([128, 4 * QB], BF16, tag="blk", bufs=6, name="sblk")
                    if split == "chunk":
                        for ci in range(4):
                            queue.dma_start(
                                out=t[:, ci * QB : (ci + 1) * QB],
                                in_=x_dram[
                                    (cs * 4 + ci) * 128 : (cs * 4 + ci + 1) * 128,
                                    n * QB : (n + 1) * QB,
                                ],
                            )
                    elif split:
                        for ci in (0, 2):
                            queue.dma_start(
                                out=t[:, ci * QB : (ci + 2) * QB].rearrange(
                                    "p (c q) -> p c q", q=QB
                                ),
                                in_=x_dram[
                                    (cs * 4 + ci) * 128 : (cs * 4 + ci + 2) * 128,
                                    n * QB : (n + 1) * QB,
                                ].rearrange("(c p) q -> p c q", p=128),
                            )
                    else:
                        queue.dma_start(
                            out=t.rearrange("p (c q) -> p c q", q=QB),
                            in_=x_dram[
                                cs * 512 : (cs + 1) * 512, n * QB : (n + 1) * QB
                            ].rearrange("(c p) q -> p c q", p=128),
                        )
                    return t

                xq = []
                head_sblk = {}
                # x-queue alternation phase: pass0-n2 stays on sync (a FIFO
                # continuation of the head; the scalar queue's first-use init
                # would land right at the n0->n2 seam), scalar starts at n3
                # where its init is absorbed in slack.
                xqi = [0]
                for pass_i in range(3):
                    x_dram = [qT, kvT, kvT][pass_i]
                    dst = [hq_sb, hk_sb, hv_sb][pass_i]
                    is_v = pass_i == 2
                    if pass_i == 0:
                        # Cold-start head.  All DMA queues stripe over the same
                        # 16 physical engines (~340GB/s aggregate), so more
                        # queues add latency, not bandwidth.  Enqueue (pass0,
                        # n0) strictly in consumption order on two queues -- x
                        # chunks on sync, w chunks on gpsimd -- split per-chunk
                        # so each matmul waits on 128KB at the leading edge.
                        # 8 pair-grain x issues on sync (exactly the 8-deep
                        # completion-sem rotation; more issues stall the
                        # engine).  Measured alternatives both lose: per-chunk
                        # grain for cs0 forces cs2/cs3 to whole-superblock
                        # grain (issue budget), whose 512KB completion lumps
                        # open gaps at their consumption edges.
                        for cs in range(4):
                            head_sblk[cs] = x_sblk(x_dram, cs, 0, nc.sync, split=True)
                        for c in range(NCH):
                            w_dma(0, c)
                    if pass_i == 2:
                        # x superblocks for the deferred Q-projection of
                        # q-block 1: stream them during the V pass, on the
                        # otherwise DMA-idle scalar queue so the V pass's own
                        # x stream (sync queue) sees no extra sem counts.
                        for cs in range(4):
                            t = pp.tile(
                                [128, 4 * QB], BF16, tag=f"xq{cs}", bufs=1, name=f"xq{cs}"
                            )
                            nc.gpsimd.dma_start(
                                out=t.rearrange("p (c q) -> p c q", q=QB),
                                in_=qT[cs * 512 : (cs + 1) * 512, QB : 2 * QB].rearrange(
                                    "(c p) q -> p c q", p=128
                                ),
                            )
                            xq.append(t)
                    for n in ((0, 2, 3) if pass_i == 0 else range(NQ)):
                        # j0/j3 share one wide pp0 tile (bank-aligned halves) so
                        # every accumulator tag stays double-buffered across n.
                        acc03 = psp.tile([128, 2 * QB], F32, tag="pp0", name="acc03")
                        acc1 = psp.tile([128, QB], F32, tag="pp1", name="acc1")
                        acc2 = psp.tile([128, QB], F32, tag="pp2", name="acc2")
                        accs = [acc03[:, 0:QB], acc1[:], acc2[:], acc03[:, QB : 2 * QB]]
                        if not (pass_i == 0 and n == 0):
                            # alternate the x stream between the sync and
                            # scalar queues per block: a DMA issue can stall
                            # its engine for microseconds (8-deep completion
                            # sem rotation + tile-WAR waits on PE progress),
                            # and on one queue that stall delays the NEXT
                            # block's transfers too.
                            x_q = (nc.sync, nc.scalar)[xqi[0] % 2]
                            xqi[0] += 1
                        for cs in range(NCH // 4):
                            first = pass_i == 0 and n == 0
                            if not first:
                                # pass0-n2's first superblock also rides the
                                # n0 leading edge (n0 now finishes early), so
                                # split it too.
                                sp = pass_i == 0 and n == 2 and cs == 0
                                sblk = x_sblk(x_dram, cs, n, x_q, split=sp)
                            else:
                                sblk = head_sblk[cs]
                            for ci in range(4):
                                c = cs * 4 + ci
                                blk = sblk[:, ci * QB : (ci + 1) * QB]
                                w_sb = w_tiles[(pass_i, c)]
                                for j in range(4):
                                    if is_v:
                                        # hv[k, d]: lhsT = kv block cols, rhs = w chunk
                                        nc.tensor.matmul(
                                            accs[j][:],
                                            blk[:, j * 128 : (j + 1) * 128],
                                            w_sb[:],
                                            start=(c == 0),
                                            stop=(c == NCH - 1),
                                        )
                                    else:
                                        # hxT[d, q]: lhsT = w chunk head j, rhs = x block
                                        nc.tensor.matmul(
                                            accs[j][:],
                                            w_sb[:, j * 128 : (j + 1) * 128],
                                            blk[:],
                                            start=(c == 0),
                                            stop=(c == NCH - 1),
                                        )
                        # prefetch next pass's weights late in this pass: the
                        # gpsimd queue must be quiet around the n0->n2 seam
                        # (queues share the 16 DMA engines; an early prefetch
                        # steals bandwidth from the critical x stream).
                        if pass_i == 0 and n == 0:
                            nc.gpsimd.dma_start(out=ones_sb[:], in_=allones)
                        if (pass_i == 0 and n == 3) or (pass_i == 1 and n == 2):
                            for c in range(NCH):
                                w_dma(pass_i + 1, c)
                        # j0/j3 are the pp0 halves; copy them first, split
                        # over ACT and DVE, so the attention phase's first
                        # pp0 alloc isn't held behind a serial ACT drain.
                        for j in (0, 3, 1, 2):
                            if is_v:
                                # kt = n*4+j holds [128 k, 512(=4h x 128 d)]
                                d_sl = dst[:, (n * 4 + j) * HD : (n * 4 + j + 1) * HD]
                            else:
                                d_sl = dst[:, j * L + n * QB : j * L + (n + 1) * QB]
                            if j in (0, 1):
                                nc.scalar.activation(d_sl, accs[j][:], COPY)
                            else:
                                nc.vector.tensor_copy(out=d_sl, in_=accs[j][:])

            # ---------------- attention + Wo ----------------
            with tc.tile_pool(name="attn", bufs=1) as ap:
                wo_sb = ap.tile([128, HL * DOUT], BF16, tag="wo", bufs=1, name="wo")
                for h in range(HL):
                    nc.gpsimd.dma_start(
                        out=wo_sb[:, h * DOUT : (h + 1) * DOUT],
                        in_=woT[h * 128 : (h + 1) * 128, :],
                    )

                wo_count = [0]

                def emit_wo_group(n_, o_sb_, g):
                    # one Wo output group (qtl, m) for q block n_: 4 matmuls
                    qtl, m = divmod(g, 4)
                    ps_f = psp.tile([128, QB], F32, tag="pp2", name="ps_f")
                    for h_ in range(HL):
                        nc.tensor.matmul(
                            ps_f[:],
                            o_sb_[:, h_ * QB + qtl * 128 : h_ * QB + (qtl + 1) * 128],
                            wo_sb[:, h_ * DOUT + m * QB : h_ * DOUT + (m + 1) * QB],
                            start=(h_ == 0),
                            stop=(h_ == HL - 1),
                        )
                    stage = ap.tile([128, QB], BF16, tag="stage", bufs=4, name="stage")
                    # spread the PSUM->SBUF stage casts evenly over ACT/DVE.
                    # All output DMAs stay on the sync queue: a gpsimd-queue
                    # transfer near the end of the kernel turns the epilogue's
                    # gpsimd DRAIN into an ~8us stall.
                    if wo_count[0] % 2 == 0:
                        nc.scalar.activation(stage[:], ps_f[:], COPY)
                    else:
                        nc.vector.tensor_copy(out=stage[:], in_=ps_f[:])
                    nc.sync.dma_start(
                        out=out[
                            n_ * QB + qtl * 128 : n_ * QB + (qtl + 1) * 128,
                            m * QB : (m + 1) * QB,
                        ],
                        in_=stage[:],
                    )
                    wo_count[0] += 1

                def flush_fold(st):
                    # deferred normalization, part 1: fold partitions on PE,
                    # reciprocal on DVE.
                    ps_o_, tr0_, o_sb_, h_ = st
                    foldt = psp.tile([128, 2 * QB], F32, tag="pp0", name="fold")
                    fold = foldt[:, 0:QB]
                    nc.tensor.matmul(
                        fold, ones_sb[:], tr0_[:, 0:QB], start=True, stop=True
                    )
                    recip = ap.tile([128, QB], F32, tag="recip", bufs=2, name="recip")
                    nc.vector.reciprocal_approx_fast(out=recip[:], in_=fold)
                    return recip

                def flush_mul(st, recip):
                    # part 2: the scale into o_sb.  Cross-engine deps are
                    # tile-granular, so every o_sb read emitted after this
                    # waits on it -- emit it as late as the real consumers
                    # allow.
                    ps_o_, tr0_, o_sb_, h_ = st
                    nc.vector.tensor_mul(
                        out=o_sb_[:, h_ * QB : (h_ + 1) * QB],
                        in0=ps_o_[:],
                        in1=recip[:],
                    )

                def flush(st):
                    # deferred normalization of the previous (n, h) iteration;
                    # runs mid-next-iteration so the PE never waits on DVE's
                    # tree.
                    flush_mul(st, flush_fold(st))

                # Wo slots per head-iteration: (n, 0) slots start late so the
                # previous block's last normalization chain has landed.
                WO_SLOTS = {0: (5, 7, 9), 1: (1, 3, 5, 7, 9), 2: (3, 5, 7, 9), 3: (3, 5, 7, 9)}
                pending = None
                o_tiles = {}
                for n in range(NQ):
                    o_sb = ap.tile([128, HL * QB], BF16, tag="o", bufs=2, name="o")
                    o_tiles[n] = o_sb
                    gi = [0]
                    for h in range(HL):
                        last_iter = n == NQ - 1 and h == HL - 1
                        hq_sl = hq_sb[:, h * L + n * QB : h * L + (n + 1) * QB]
                        ps_o = psp.tile([128, QB], F32, tag="pp1", name="ps_o")
                        if n == 0:
                            # deferred Q-projection: head h of q-block 1
                            acc_q = psp.tile([128, QB], F32, tag="pp2", name="acc_q")
                        tr = [
                            ap.tile([128, 4 * QB], BF16, tag=f"tr{i}", bufs=2, name=f"tr{i}")
                            for i in range(2)
                        ]
                        exp_half = [None, None]

                        def e_sl(kt):
                            return exp_half[kt // 8][:, (kt % 8) * QB : (kt % 8 + 1) * QB]

                        def tree(i, lvl):
                            # binary-tree partial sums of exp_half[i] on DVE
                            w = (4 >> lvl) * QB
                            src = exp_half[i] if lvl == 0 else tr[i]
                            nc.vector.tensor_add(
                                out=tr[i][:, 0:w], in0=src[:, 0:w], in1=src[:, w : 2 * w]
                            )

                        # 11 steps: scores/exp for pair p (p<8), AV lagged
                        # three pairs (p>=3) so ACT's exp stays off the PE
                        # critical path.  Softmax denominator: DVE tree-sums
                        # the bf16 exp tiles; the PE partition-fold for the
                        # PREVIOUS iteration is slotted in at p==2.
                        for p in range(11):
                            if p < 8:
                                half = p // 4
                                if p % 4 == 0:
                                    exp_half[half] = ap.tile(
                                        [128, 8 * QB], BF16, tag="exp", bufs=3, name="exp"
                                    )
                                off = (p % 4) * 2 * QB
                                ps_s = psp.tile([128, 2 * QB], F32, tag="pp0", name="ps_s")
                                for t in range(2):
                                    kt = 2 * p + t
                                    nc.tensor.matmul(
                                        ps_s[:, t * QB : (t + 1) * QB],
                                        hk_sb[:, h * L + kt * 128 : h * L + (kt + 1) * 128],
                                        hq_sl,
                                        start=True,
                                        stop=True,
                                    )
                                nc.scalar.activation(
                                    exp_half[half][:, off : off + 2 * QB], ps_s[:], EXP
                                )
                            if p >= 3:
                                for t in range(2):
                                    kt = 2 * (p - 3) + t
                                    nc.tensor.matmul(
                                        ps_o[:],
                                        hv_sb[:, kt * HD + h * 128 : kt * HD + (h + 1) * 128],
                                        e_sl(kt),
                                        start=(kt == 0),
                                        stop=(kt == NKT - 1),
                                    )
                            if p == 2 and pending is not None:
                                flush(pending)
                                pending = None
                            if p in (5, 6, 7):
                                tree(0, p - 5)
                            # second-half tree split so the tiles gated on the
                            # last exp pair form the shortest possible chain
                            if p == 6:
                                # tiles 8-11 (pairs 4,5 exp'd by now)
                                nc.vector.tensor_add(
                                    out=tr[1][:, 0 : 2 * QB],
                                    in0=exp_half[1][:, 0 : 2 * QB],
                                    in1=exp_half[1][:, 2 * QB : 4 * QB],
                                )
                            elif p == 7:
                                nc.vector.tensor_add(
                                    out=tr[1][:, 0:QB],
                                    in0=tr[1][:, 0:QB],
                                    in1=tr[1][:, QB : 2 * QB],
                                )
                            elif p == 8 and not last_iter:
                                # tiles 12-15 (pairs 6,7)
                                nc.vector.tensor_add(
                                    out=tr[1][:, 2 * QB : 4 * QB],
                                    in0=exp_half[1][:, 4 * QB : 6 * QB],
                                    in1=exp_half[1][:, 6 * QB : 8 * QB],
                                )
                            elif p == 9 and not last_iter:
                                nc.vector.tensor_add(
                                    out=tr[1][:, 2 * QB : 3 * QB],
                                    in0=tr[1][:, 2 * QB : 3 * QB],
                                    in1=tr[1][:, 3 * QB : 4 * QB],
                                )
                                nc.vector.tensor_add(
                                    out=tr[1][:, 0:QB],
                                    in0=tr[1][:, 0:QB],
                                    in1=tr[1][:, 2 * QB : 3 * QB],
                                )
                                nc.vector.tensor_add(
                                    out=tr[0][:, 0:QB],
                                    in0=tr[0][:, 0:QB],
                                    in1=tr[1][:, 0:QB],
                                )
                            elif p == 8 and last_iter:
                                # final iteration: the tail can't start its h3
                                # work until this denominator lands, so fold
                                # everything not gated on the LAST exp pair in
                                # now (depth after the last exp: 2 adds, not 4)
                                nc.vector.tensor_add(
                                    out=tr[1][:, 2 * QB : 3 * QB],
                                    in0=exp_half[1][:, 4 * QB : 5 * QB],
                                    in1=exp_half[1][:, 5 * QB : 6 * QB],
                                )
                                nc.vector.tensor_add(
                                    out=tr[1][:, 0:QB],
                                    in0=tr[1][:, 0:QB],
                                    in1=tr[1][:, 2 * QB : 3 * QB],
                                )
                                nc.vector.tensor_add(
                                    out=tr[0][:, 0:QB],
                                    in0=tr[0][:, 0:QB],
                                    in1=tr[1][:, 0:QB],
                                )
                            elif p == 9 and last_iter:
                                nc.vector.tensor_add(
                                    out=tr[1][:, 3 * QB : 4 * QB],
                                    in0=exp_half[1][:, 6 * QB : 7 * QB],
                                    in1=exp_half[1][:, 7 * QB : 8 * QB],
                                )
                                nc.vector.tensor_add(
                                    out=tr[0][:, 0:QB],
                                    in0=tr[0][:, 0:QB],
                                    in1=tr[1][:, 3 * QB : 4 * QB],
                                )
                            if n == 0:
                                if p in (3, 5, 7, 9):
                                    # 4 deferred Q-projection matmuls per slot
                                    si = (p - 3) // 2
                                    for ci in range(4):
                                        c = si * 4 + ci
                                        nc.tensor.matmul(
                                            acc_q[:],
                                            w_tiles[(0, c)][:, h * 128 : (h + 1) * 128],
                                            xq[si][:, ci * QB : (ci + 1) * QB],
                                            start=(c == 0),
                                            stop=(c == NCH - 1),
                                        )
                            elif p in WO_SLOTS[h]:
                                emit_wo_group(n - 1, o_tiles[n - 1], gi[0])
                                gi[0] += 1
                        if n == 0:
                            # hq for q-block 1, head h (read from (1, h) on)
                            nc.vector.tensor_copy(
                                out=hq_sb[:, h * L + QB : h * L + 2 * QB],
                                in_=acc_q[:],
                            )
                        pending = (ps_o, tr[0], o_sb, h)
                    if n > 0:
                        o_tiles.pop(n - 1)
                # tail: the last block's Wo runs with no scores left.  The
                # last head's softmax flush (DVE tree -> PE fold -> recip ->
                # mul) is still in flight when the tail starts, and the PE
                # stream is in-order, so: 512-wide groups whose h3 matmul
                # trails two groups behind the h0-h2 accumulation, with the
                # fold slotted after group 1's head.  One-bank accumulators
                # alternate over the pp2/pp1 tags (4 tiles in flight = 3.5us
                # of WAR slack vs the ~0.7us stage-cast + DMA drain; 2-bank
                # pp0 supergroups stall the PE on that drain every other
                # group).  Stage casts alternate ACT/DVE and output DMAs
                # alternate sync/gpsimd.
                o_last = o_tiles.pop(NQ - 1)
                n_ = NQ - 1
                ps_tails = {}

                def g_head(g):
                    qtl, m = divmod(g, 4)
                    ps_f = psp.tile(
                        [128, QB], F32, tag=("pp2", "pp1", "pp0")[g % 3], name="ps_tail"
                    )
                    ps_tails[g] = ps_f
                    for h_ in range(HL - 1):
                        nc.tensor.matmul(
                            ps_f[:],
                            o_last[:, h_ * QB + qtl * 128 : h_ * QB + (qtl + 1) * 128],
                            wo_sb[:, h_ * DOUT + m * QB : h_ * DOUT + (m + 1) * QB],
                            start=(h_ == 0),
                            stop=False,
                        )

                def g_tail(g, split=False):
                    qtl, m = divmod(g, 4)
                    ps_f = ps_tails.pop(g)
                    h_ = HL - 1
                    nc.tensor.matmul(
                        ps_f[:],
                        o_last[:, h_ * QB + qtl * 128 : h_ * QB + (qtl + 1) * 128],
                        wo_sb[:, h_ * DOUT + m * QB : h_ * DOUT + (m + 1) * QB],
                        start=False,
                        stop=True,
                    )
                    stage = ap.tile([128, QB], BF16, tag="stage", bufs=4, name="stage")
                    o_sl = out[
                        n_ * QB + qtl * 128 : n_ * QB + (qtl + 1) * 128,
                        m * QB : (m + 1) * QB,
                    ]
                    if split:
                        # last trailing groups: half casts on BOTH engines in
                        # parallel + half DMAs, so the final write (which
                        # gates the epilogue's queue-drain barrier) lands
                        # right behind the last matmul
                        hw = QB // 2
                        nc.scalar.activation(stage[:, 0:hw], ps_f[:, 0:hw], COPY)
                        nc.vector.tensor_copy(out=stage[:, hw:QB], in_=ps_f[:, hw:QB])
                        nc.sync.dma_start(out=o_sl[:, 0:hw], in_=stage[:, 0:hw])
                        nc.sync.dma_start(out=o_sl[:, hw:QB], in_=stage[:, hw:QB])
                    else:
                        if g % 2 == 0:
                            nc.scalar.activation(stage[:], ps_f[:], COPY)
                        else:
                            nc.vector.tensor_copy(out=stage[:], in_=ps_f[:])
                        nc.sync.dma_start(out=o_sl[:], in_=stage[:])

                # Cross-engine deps are tile-granular: any o_last read emitted
                # after the flush's mul waits on it, so run the fold+recip
                # right away (PE reaches the fold after just 3 matmuls) but
                # defer the mul until the last pre-flush g_head -- by the time
                # the PE reaches g_tail(0)'s h3 matmul, the mul has landed.
                g_head(0)
                recip_last = flush_fold(pending)
                for g in range(1, 6):
                    g_head(g)
                flush_mul(pending, recip_last)
                for g in range(6, 16):
                    g_tail(g - 6)
                    g_head(g)
                for g in range(10, 16):
                    g_tail(g, split=g >= 13)
    nc.compile()
    return nc


def _get_nc():
    if "nc" not in _CACHE:
        _CACHE["nc"] = _build_nc()
    return _CACHE["nc"]


def make_in_maps(query, key_value, Wq, Wk, Wv, Wo):
    import ml_dtypes

    bf16 = ml_dtypes.bfloat16
    scale = 1.0 / math.sqrt(D)
    allones = np.ones((128, 128), bf16)
    in_maps = []
    qT = [np.ascontiguousarray(query[b].T.astype(bf16)) for b in range(B)]
    kvT = [np.ascontiguousarray(key_value[b].T.astype(bf16)) for b in range(B)]
    for core in range(NC_):
        b, g = divmod(core, NC_ // B)
        sl = slice(g * HD, (g + 1) * HD)
        in_maps.append(
            {
                "qT": qT[b],
                "kvT": kvT[b],
                "wqT": np.ascontiguousarray((Wq[sl, :] * scale).T.astype(bf16)),
                "wkT": np.ascontiguousarray(Wk[sl, :].T.astype(bf16)),
                "wvT": np.ascontiguousarray(Wv[sl, :].T.astype(bf16)),
                "woT": np.ascontiguousarray(Wo[:, sl].T.astype(bf16)),
                "allones": allones,
            }
        )
    return in_maps


def _numpy_fallback(query, key_value, attention_mask, Wq, Wk, Wv, Wo):
    # Only reached if the mask is not all-ones (never per the problem spec).
    q64, kv64 = query.astype(np.float64), key_value.astype(np.float64)
    hq = (q64 @ Wq.T.astype(np.float64)).reshape(B, L, NH, D).transpose(0, 2, 1, 3)
    hk = (kv64 @ Wk.T.astype(np.float64)).reshape(B, L, NH, D).transpose(0, 2, 1, 3)
    hv = (kv64 @ Wv.T.astype(np.float64)).reshape(B, L, NH, D).transpose(0, 2, 1, 3)
    s = np.einsum("bhqd,bhkd->bhqk", hq, hk) / math.sqrt(D)
    mask = attention_mask[:, None, :, :]
    s = np.where(mask, s, -np.inf)
    s = s - s.max(axis=-1, keepdims=True)
    e = np.exp(s)
    p = e / np.maximum(e.sum(axis=-1, keepdims=True), 1e-300)
    p = np.where(mask, p, 0.0)
    o = np.einsum("bhqk,bhkd->bhqd", p, hv)
    o = o.transpose(0, 2, 1, 3).reshape(B, L, NH * D)
    return (o @ Wo.T.astype(np.float64)).astype(np.float32)


def kernel(query, key_value, attention_mask, Wq, Wk, Wv, Wo):
    query = np.asarray(query)
    key_value = np.asarray(key_value)
    attention_mask = np.asarray(attention_mask)
    Wq, Wk, Wv, Wo = (np.asarray(a) for a in (Wq, Wk, Wv, Wo))

    if not attention_mask.all():
        return _numpy_fallback(query, key_value, attention_mask, Wq, Wk, Wv, Wo)

    from concourse.bass_utils import run_bass_kernel_spmd

    nc = _get_nc()
    in_maps = make_in_maps(query, key_value, Wq, Wk, Wv, Wo)
    res = run_bass_kernel_spmd(nc, in_maps, list(range(NC_))).results
    out = np.zeros((B, L, DOUT), np.float32)
    for core in range(NC_):
        b = core // (NC_ // B)
        out[b] += np.asarray(res[core]["out"]).astype(np.float32)
    return out

